# revision 9
# baseline (speedup 1.0000x reference)
"""Trainium2 Bass kernel for the MiniGRU cell (B=131072 rows, data-parallel over 8 cores).

Math (per row b):
    tokens = concat(stoch, action) @ proj_w + proj_b            # [256]
    parts  = LN(concat(tokens, deter) @ core_w) * g + b         # [768]
    reset, cand_in, upd_in = split(parts, 3)
    reset = sigmoid(reset); cand = tanh(reset * cand_in); upd = sigmoid(upd_in - 1)
    out = upd * cand + (1 - upd) * deter                        # [256]

Host-side folding: both matmuls collapse into one x_aug @ W_c where
x_aug = [stoch, deter, action, 1] (417 features, zero-padded to 512) and
W_c has its per-row column-mean removed so the LayerNorm mean subtraction
is built into the matmul (mean(q) == 0 up to rounding); the device only
computes rstd = 1/sqrt(mean(q^2) + eps) (RMS-style).  W_c is additionally
Frobenius-normalized so mean(q^2) ~= 1, which lets a 2-step Newton rsqrt
(from y0=1) replace the ACT Sqrt table (whose set excludes Sigmoid/Tanh).

Device layout: activations are fed feature-major (host pre-transposed) as the
matmul's stationary operand, weights stream as the moving operand, so the
matmul output lands batch-major in PSUM ([128 rows, 768]) where LN stats are a
free-axis reduction.

Engine balance (the previous version bottlenecked on GPSIMD, which runs
2-input elementwise ops at ~0.42 of DVE speed):
  - DVE:  bn_stats over q[:, 0:512], Newton-rsqrt chain (pair-batched),
          tt1 = (q_c * rstd) * sig_r as one fused scalar_tensor_tensor
          (also evacuates the cand third of PSUM), final out = d2 + det
          in bf16 (2x mode).
  - ACT:  Square(q_u) with accum_out completes the sum-of-squares for the
          update third; both sigmoids read PSUM directly with scale=rstd
          (evacuation fused); tanh batched per tile-pair.  All functions
          (Square/Sigmoid/Tanh) live in the sigmoid_and_others table set.
  - Pool: only d1 = cand - det and d2 = upd * d1, pair-batched.
  - Output is written bf16 and upcast to f32 on the host.
Emission is software-pipelined (tanh/pool lag one pair-section, the final
add + store lag two) so no engine FIFO head-of-line blocks another engine.
"""

import os
import sys

for _p in ("/opt/trn_rl_repo",):
    if _p not in sys.path and os.path.isdir(_p):
        sys.path.insert(0, _p)

import numpy as np
import ml_dtypes

from contextlib import ExitStack

import concourse.bass as bass
import concourse.bacc as bacc
import concourse.tile as tile
from concourse import mybir
from concourse.bass_utils import run_bass_kernel_spmd

BF16 = ml_dtypes.bfloat16

B_FULL = 131072
DETER = 256
STOCH = 128
ACT_DIM = 16
HID = 256
NOUT = 3 * DETER          # 768
N_CORES = 8
BC = B_FULL // N_CORES    # 16384 rows per core
KPAD = 512                # padded contraction dim: [stoch 128 | deter 256 | action 16 | ones 1 | zeros 111]
LN_EPS = 1e-5

OCT = 1024                # batch rows per DMA slab (8 tiles of 128 = 4 pairs)
N_OCT = BC // OCT         # 16

_F32 = mybir.dt.float32
_BF16 = mybir.dt.bfloat16

# Matmul N-chunks (offset, n): 768 outputs as 512+256, each within one PSUM bank.
_CHUNKS = ((0, 512), (512, 256))

# LN stats split: DVE bn_stats covers q[:, :BN_N], ACT Square-accum covers the rest.
BN_N = 512
SQ_N = NOUT - BN_N        # 256

_last_results = None  # BassKernelResults of the most recent run (for profiling)


def build_nc(bc: int = BC, loop: int = 1) -> bass.Bass:
    """Build the per-core Bass program. All 8 cores run this same program.

    `loop` > 1 wraps the whole per-core body in a hardware For_i that re-runs
    it `loop` times over the same data — used only by the timing bench.
    """
    n_oct = bc // OCT
    nc = bacc.Bacc("TRN2", target_bir_lowering=False, debug=False, num_devices=1)

    xw = nc.declare_dram_parameter("xw", [128, 4, bc], _BF16, isOutput=False)
    wts = nc.declare_dram_parameter("wts", [128, 4, NOUT], _BF16, isOutput=False)
    det = nc.declare_dram_parameter("det", [bc, DETER], _BF16, isOutput=False)
    out = nc.declare_dram_parameter("out", [bc, DETER], _BF16, isOutput=True)

    with tile.TileContext(nc) as tc, ExitStack() as ctx:
        singles = ctx.enter_context(tc.tile_pool(name="singles", bufs=1))
        xpool = ctx.enter_context(tc.tile_pool(name="x", bufs=2))
        dpool = ctx.enter_context(tc.tile_pool(name="det", bufs=2))
        gpool = ctx.enter_context(tc.tile_pool(name="gates", bufs=4))
        spool = ctx.enter_context(tc.tile_pool(name="stats", bufs=4))
        qpool = ctx.enter_context(tc.tile_pool(name="q", bufs=4, space="PSUM"))

        w_t = singles.tile([128, 4, NOUT], _BF16)
        nc.sync.dma_start(w_t[:], wts[:])
        neg1_t = singles.tile([128, 1], _F32)
        nc.vector.memset(neg1_t[:], -1.0)

        pools = dict(xpool=xpool, dpool=dpool, gpool=gpool, spool=spool, qpool=qpool,
                     neg1=neg1_t)

        def body():
            st = _PipeState()
            for o in range(n_oct):
                oct_body(nc, pools, w_t, xw, det, out, o, st)
            flush(nc, out, st)

        if loop > 1:
            with tc.For_i(0, loop):
                body()
        else:
            body()

    nc.finalize()
    return nc


class _PipeState:
    """Deferred pair-sections for software pipelining across emission."""

    def __init__(self):
        self.stage_b = []  # pairs awaiting tanh + pool d1/d2
        self.stage_c = []  # pairs awaiting final add + store


def oct_body(nc, pools, w_t, xw, det, out, o, st):
    """One oct (1024 batch rows = 4 pair-sections of 2x128 rows)."""
    xpool, dpool, gpool, spool, qpool = (
        pools["xpool"], pools["dpool"], pools["gpool"], pools["spool"], pools["qpool"])

    x_t = xpool.tile([128, 4, OCT], _BF16, name=f"x_{o}", tag="x")
    nc.sync.dma_start(x_t[:], xw[:, :, o * OCT:(o + 1) * OCT])
    det_t = dpool.tile([128, 8, DETER], _BF16, name=f"dt_{o}", tag="det")
    nc.sync.dma_start(
        det_t[:], det[o * OCT:(o + 1) * OCT, :].rearrange("(t p) f -> p t f", p=128)
    )

    for p in range(4):
        # ---- stage B of the previous pair: tanh first so ACT never stalls on it
        if st.stage_b:
            emit_tanh(nc, st.stage_b[0])

        # ---- stage A(p): matmuls + stats + rstd + gate evacuations
        qts = []
        sqa = spool.tile([128, 2, 1], _F32, name=f"sqa_{o}_{p}", tag="sqa")
        st6 = spool.tile([128, 2, 6], _F32, name=f"st6_{o}_{p}", tag="st6")
        mv = spool.tile([128, 2, 2], _F32, name=f"mv_{o}_{p}", tag="mv")
        sqs = gpool.tile([128, 2, SQ_N], _BF16, name=f"sqs_{o}_{p}", tag="sqs")
        for j in range(2):
            t = 2 * p + j
            qt = qpool.tile([128, NOUT], _F32, name=f"q{o}_{t}", tag="q")  # 2 banks
            qts.append(qt)
            lhs_cols = slice(t * 128, (t + 1) * 128)
            for k in range(4):
                for (qo, n) in _CHUNKS:
                    nc.tensor.matmul(
                        qt[:, qo:qo + n],
                        x_t[:, k, lhs_cols],
                        w_t[:, k, qo:qo + n],
                        start=(k == 0),
                        stop=(k == 3),
                    )
            nc.vector.bn_stats(st6[:, j, :], qt[:, 0:BN_N])
            nc.scalar.activation(
                out=sqs[:, j], in_=qt[:, BN_N:NOUT],
                func=mybir.ActivationFunctionType.Square,
                accum_out=sqa[:, j, :],
            )
            nc.vector.bn_aggr(mv[:, j, :], st6[:, j, :])

        # rstd for the pair: v = (BN_N*(var+mu^2) + sqa)/768 + eps, then
        # 2-step Newton rsqrt from y0=1 (W is normalized so v ~= 1):
        #   y1 = 1.5 - 0.5 v;  rstd = y1*(1.5 - 0.5*v*y1^2)
        m2 = spool.tile([128, 2], _F32, name=f"m2_{o}_{p}", tag="m2")
        v = spool.tile([128, 2], _F32, name=f"v_{o}_{p}", tag="v")
        y1 = spool.tile([128, 2], _F32, name=f"y1_{o}_{p}", tag="y1")
        t2 = spool.tile([128, 2], _F32, name=f"t2_{o}_{p}", tag="t2")
        rstd = spool.tile([128, 2], _F32, name=f"rstd_{o}_{p}", tag="rstd")
        mean = mv[:, :, 0]
        var = mv[:, :, 1]
        sqaf = sqa[:, :, 0]
        nc.vector.tensor_mul(m2[:], mean, mean)                      # mu^2
        nc.vector.tensor_add(m2[:], m2[:], var)                      # mu^2 + var
        nc.vector.tensor_scalar(
            out=v[:], in0=sqaf, scalar1=1.0 / NOUT, scalar2=LN_EPS,
            op0=mybir.AluOpType.mult, op1=mybir.AluOpType.add,
        )                                                            # sqa/768 + eps
        nc.vector.scalar_tensor_tensor(
            out=v[:], in0=m2[:], scalar=float(BN_N) / NOUT, in1=v[:],
            op0=mybir.AluOpType.mult, op1=mybir.AluOpType.add,
        )                                                            # + BN_N*(..)/768
        nc.vector.tensor_scalar(
            out=y1[:], in0=v[:], scalar1=-0.5, scalar2=1.5,
            op0=mybir.AluOpType.mult, op1=mybir.AluOpType.add,
        )
        nc.vector.tensor_mul(t2[:], y1[:], y1[:])                    # y1^2
        nc.vector.tensor_mul(t2[:], v[:], t2[:])                     # v*y1^2
        nc.vector.tensor_scalar(
            out=t2[:], in0=t2[:], scalar1=-0.5, scalar2=1.5,
            op0=mybir.AluOpType.mult, op1=mybir.AluOpType.add,
        )
        nc.vector.tensor_mul(rstd[:], t2[:], y1[:])

        # ---- stage C of the pair before last: final add + store (on DVE; fills
        # the gap while ACT waits for rstd)
        if st.stage_c:
            emit_out(nc, out, st.stage_c.pop(0))

        # ---- stage A(p) continued: gate evacuations
        sig_r = gpool.tile([128, 2, DETER], _BF16, name=f"sig_{o}_{p}", tag="sig")
        upd = gpool.tile([128, 2, DETER], _BF16, name=f"upd_{o}_{p}", tag="upd")
        tt1 = gpool.tile([128, 2, DETER], _BF16, name=f"tt1_{o}_{p}", tag="tt1")
        for j in range(2):
            r_ = rstd[:, j:j + 1]
            nc.scalar.activation(
                out=sig_r[:, j], in_=qts[j][:, 0:DETER],
                func=mybir.ActivationFunctionType.Sigmoid, scale=r_,
            )
            nc.scalar.activation(
                out=upd[:, j], in_=qts[j][:, 2 * DETER:3 * DETER],
                func=mybir.ActivationFunctionType.Sigmoid, scale=r_,
                bias=pools["neg1"][:],
            )
        for j in range(2):
            nc.vector.scalar_tensor_tensor(
                out=tt1[:, j], in0=qts[j][:, DETER:2 * DETER],
                scalar=rstd[:, j:j + 1], in1=sig_r[:, j],
                op0=mybir.AluOpType.mult, op1=mybir.AluOpType.mult,
            )                                                        # (q_c*rstd)*sig_r

        # ---- stage B(p-1) continued: pool products (then queue for stage C)
        if st.stage_b:
            s = st.stage_b.pop(0)
            emit_pool(nc, s)
            st.stage_c.append(s)

        st.stage_b.append(dict(
            o=o, p=p, tt1=tt1, upd=upd, det_t=det_t, gpool=gpool))

    # oct ends; stages drain via subsequent sections (or flush at the end)


def emit_tanh(nc, s):
    gpool = s["gpool"]
    cand = gpool.tile([128, 2, DETER], _BF16, name=f"cand_{s['o']}_{s['p']}", tag="cand")
    s["cand"] = cand
    nc.scalar.activation(
        out=cand[:], in_=s["tt1"][:], func=mybir.ActivationFunctionType.Tanh,
    )


def emit_pool(nc, s):
    gpool = s["gpool"]
    p = s["p"]
    d1 = gpool.tile([128, 2, DETER], _BF16, name=f"d1_{s['o']}_{p}", tag="d1")
    d2 = gpool.tile([128, 2, DETER], _BF16, name=f"d2_{s['o']}_{p}", tag="d2")
    dslc = s["det_t"][:, 2 * p:2 * p + 2]
    nc.gpsimd.tensor_sub(d1[:], s["cand"][:], dslc)                  # cand - det
    nc.gpsimd.tensor_mul(d2[:], s["upd"][:], d1[:])                  # upd * (cand - det)
    s["d2"] = d2
    s["dslc"] = dslc


def emit_out(nc, out, s):
    gpool = s["gpool"]
    o, p = s["o"], s["p"]
    outb = gpool.tile([128, 2, DETER], _BF16, name=f"outb_{o}_{p}", tag="outb")
    nc.vector.tensor_add(outb[:], s["d2"][:], s["dslc"])             # + det
    base = o * OCT + p * 256
    nc.sync.dma_start(
        out[base:base + 256, :].rearrange("(t p) f -> p t f", p=128),
        outb[:],
    )


def flush(nc, out, st):
    """Drain the deferred pipeline stages at the end of the program."""
    while st.stage_b or st.stage_c:
        if st.stage_b:
            s = st.stage_b.pop(0)
            emit_tanh(nc, s)
            emit_pool(nc, s)
            st.stage_c.append(s)
        if st.stage_c:
            emit_out(nc, out, st.stage_c.pop(0))


_nc_cache: dict[tuple, bass.Bass] = {}


def _get_nc(bc: int) -> bass.Bass:
    if (bc,) not in _nc_cache:
        _nc_cache[(bc,)] = build_nc(bc)
    return _nc_cache[(bc,)]


def _fold_weights(proj_w, proj_b, core_w):
    """Collapse both matmuls + LN mean-subtraction into one [KPAD, 768] matrix,
    Frobenius-normalized so mean over the 768 outputs of q^2 is ~1."""
    W1 = proj_w.astype(np.float64) @ core_w[:HID].astype(np.float64)   # [144, 768]
    W2 = core_w[HID:].astype(np.float64)                               # [256, 768]
    b1 = proj_b.astype(np.float64) @ core_w[:HID].astype(np.float64)   # [768]
    W_all = np.zeros((KPAD, NOUT), np.float64)
    W_all[0:STOCH] = W1[:STOCH]
    W_all[STOCH:STOCH + DETER] = W2
    W_all[STOCH + DETER:STOCH + DETER + ACT_DIM] = W1[STOCH:]
    W_all[STOCH + DETER + ACT_DIM] = b1
    # remove per-row column mean -> mean_j(x @ W_c) == 0 exactly
    W_c = W_all - W_all.mean(axis=1, keepdims=True)
    # normalize so E[mean_j q_j^2] == 1 (Newton rsqrt converges from y0=1;
    # LayerNorm output is invariant to this scale)
    W_c *= np.sqrt(NOUT / np.square(W_c).sum())
    return W_c


def kernel(deter, stoch, action, proj_w, proj_b, core_w, ln_g, ln_b):
    global _last_results
    deter = np.asarray(deter, np.float32)
    stoch = np.asarray(stoch, np.float32)
    action = np.asarray(action, np.float32)
    proj_w = np.asarray(proj_w, np.float32)
    proj_b = np.asarray(proj_b, np.float32)
    core_w = np.asarray(core_w, np.float32)
    ln_g = np.asarray(ln_g, np.float32)
    ln_b = np.asarray(ln_b, np.float32)

    if not (np.allclose(ln_g, 1.0) and np.allclose(ln_b, 0.0)):
        # General-affine LN is not wired into the device fast path; fall back to
        # exact host math (setup_inputs always passes g=1, b=0 so this is unused).
        return _host_reference(deter, stoch, action, proj_w, proj_b, core_w, ln_g, ln_b)

    B = deter.shape[0]
    assert B % N_CORES == 0
    bc = B // N_CORES

    W_c = _fold_weights(proj_w, proj_b, core_w)
    wp = np.ascontiguousarray(
        W_c.reshape(4, 128, NOUT).transpose(1, 0, 2)).astype(BF16)  # [128, 4, 768]

    # Feature-major activations, padded to KPAD rows: [stoch; deter; action; ones; zeros]
    xb = np.empty((KPAD, B), BF16)
    xb[0:STOCH] = stoch.T
    xb[STOCH:STOCH + DETER] = deter.T
    xb[STOCH + DETER:STOCH + DETER + ACT_DIM] = action.T
    xb[STOCH + DETER + ACT_DIM] = 1.0
    xb[STOCH + DETER + ACT_DIM + 1:] = 0.0
    xb = np.ascontiguousarray(xb.reshape(4, 128, B).transpose(1, 0, 2))  # [128, 4, B]

    det_b = deter.astype(BF16)

    in_maps = []
    for c in range(N_CORES):
        in_maps.append({
            "xw": np.ascontiguousarray(xb[:, :, c * bc:(c + 1) * bc]),
            "wts": wp,
            "det": np.ascontiguousarray(det_b[c * bc:(c + 1) * bc]),
        })

    nc = _get_nc(bc)
    res = run_bass_kernel_spmd(nc, in_maps, core_ids=list(range(N_CORES)))
    _last_results = res
    return np.concatenate(
        [res.results[c]["out"] for c in range(N_CORES)], axis=0
    ).astype(np.float32)


def _host_reference(deter, stoch, action, proj_w, proj_b, core_w, ln_g, ln_b):
    x = np.concatenate([stoch, action], axis=-1) @ proj_w + proj_b
    parts = np.concatenate([x, deter], axis=-1) @ core_w
    mu = parts.mean(-1, keepdims=True)
    var = ((parts - mu) ** 2).mean(-1, keepdims=True)
    parts = (parts - mu) / np.sqrt(var + LN_EPS) * ln_g + ln_b
    d = parts.shape[-1] // 3
    reset = 1.0 / (1.0 + np.exp(-parts[..., :d]))
    cand = np.tanh(reset * parts[..., d:2 * d])
    upd = 1.0 / (1.0 + np.exp(-(parts[..., 2 * d:] - 1.0)))
    return (upd * cand + (1.0 - upd) * deter).astype(np.float32)


# revision 28
# speedup vs baseline: 12.9503x; 12.9503x over previous
"""Trainium2 Bass kernel for the MiniGRU cell (B=131072 rows, data-parallel over 8 cores).

Math (per row b):
    tokens = concat(stoch, action) @ proj_w + proj_b            # [256]
    parts  = LN(concat(tokens, deter) @ core_w) * g + b         # [768]
    reset, cand_in, upd_in = split(parts, 3)
    reset = sigmoid(reset); cand = tanh(reset * cand_in); upd = sigmoid(upd_in - 1)
    out = upd * cand + (1 - upd) * deter                        # [256]

Host-side folding: both matmuls collapse into one x_aug @ W_c where
x_aug = [stoch, deter, action, 1] (417 features, zero-padded to 512) and
W_c has its per-row column-mean removed so the LayerNorm mean subtraction
is built into the matmul (mean(q) == 0 up to rounding); the device only
computes rstd = 1/sqrt(mean(q^2) + eps) (RMS-style).  W_c is additionally
Frobenius-normalized so mean(q^2) ~= 1, which lets a 2-step Newton rsqrt
(from y0=1) replace the ACT Sqrt table (whose set excludes Sigmoid/Tanh).

Device layout: activations are fed feature-major (host pre-transposed) as the
matmul's stationary operand, weights stream as the moving operand, so the
matmul output lands batch-major in PSUM ([128 rows, 768]) where LN stats are a
free-axis reduction.

Engine balance (the previous version bottlenecked on GPSIMD, which runs
2-input elementwise ops at ~0.42 of DVE speed):
  - DVE:  bn_stats over q[:, 0:512], Newton-rsqrt chain (pair-batched),
          tt1 = (q_c * rstd) * sig_r as one fused scalar_tensor_tensor
          (also evacuates the cand third of PSUM), final out = d2 + det
          in bf16 (2x mode).
  - ACT:  Square(q_u) with accum_out completes the sum-of-squares for the
          update third; both sigmoids read PSUM directly with scale=rstd
          (evacuation fused); tanh batched per tile-pair.  All functions
          (Square/Sigmoid/Tanh) live in the sigmoid_and_others table set.
  - Pool: only d1 = cand - det and d2 = upd * d1, pair-batched.
  - Output is written bf16 and upcast to f32 on the host.
Emission is software-pipelined (tanh/pool lag one pair-section, the final
add + store lag two) so no engine FIFO head-of-line blocks another engine.
"""

import os
import sys

for _p in ("/opt/trn_rl_repo",):
    if _p not in sys.path and os.path.isdir(_p):
        sys.path.insert(0, _p)

import numpy as np
import ml_dtypes

from contextlib import ExitStack

import concourse.bass as bass
import concourse.bacc as bacc
import concourse.tile as tile
from concourse import mybir
from concourse.bass_utils import run_bass_kernel_spmd

BF16 = ml_dtypes.bfloat16

B_FULL = 131072
DETER = 256
STOCH = 128
ACT_DIM = 16
HID = 256
NOUT = 3 * DETER          # 768
N_CORES = 8
BC = B_FULL // N_CORES    # 16384 rows per core
KPAD = 512                # padded contraction dim: [stoch 128 | deter 256 | action 16 | ones 1 | zeros 111]
LN_EPS = 1e-5

OCT = 1024                # batch rows per DMA slab (8 tiles of 128 = 4 pairs)
N_OCT = BC // OCT         # 16

_F32 = mybir.dt.float32
_BF16 = mybir.dt.bfloat16

# Matmul N-chunks (offset, n): 768 outputs as 512+256, each within one PSUM bank.
_CHUNKS = ((0, 512), (512, 256))

# LN stats split: DVE bn_stats covers q[:, :BN_N], ACT Square-accum covers the rest.
BN_N = 512
SQ_N = NOUT - BN_N        # 256

_last_results = None  # BassKernelResults of the most recent run (for profiling)


def build_nc(bc: int = BC, loop: int = 1) -> bass.Bass:
    """Build the per-core Bass program. All 8 cores run this same program.

    `loop` > 1 wraps the whole per-core body in a hardware For_i that re-runs
    it `loop` times over the same data — used only by the timing bench.
    """
    n_oct = bc // OCT
    nc = bacc.Bacc("TRN2", target_bir_lowering=False, debug=False, num_devices=1)

    xw = nc.declare_dram_parameter("xw", [128, 4, bc], _BF16, isOutput=False)
    wts = nc.declare_dram_parameter("wts", [128, 4, NOUT], _BF16, isOutput=False)
    det = nc.declare_dram_parameter("det", [bc, DETER], _BF16, isOutput=False)
    out = nc.declare_dram_parameter("out", [bc, DETER], _BF16, isOutput=True)

    with tile.TileContext(nc) as tc, ExitStack() as ctx:
        singles = ctx.enter_context(tc.tile_pool(name="singles", bufs=1))
        xpool = ctx.enter_context(tc.tile_pool(name="x", bufs=3))
        dpool = ctx.enter_context(tc.tile_pool(name="det", bufs=3))
        gpool = ctx.enter_context(tc.tile_pool(name="gates", bufs=4))
        spool = ctx.enter_context(tc.tile_pool(name="stats", bufs=4))
        qpool = ctx.enter_context(tc.tile_pool(name="q", bufs=4, space="PSUM"))

        w_t = singles.tile([128, 4, NOUT], _BF16)
        nc.sync.dma_start(w_t[:], wts[:])
        neg1_t = singles.tile([128, 1], _F32)
        nc.vector.memset(neg1_t[:], -1.0)

        pools = dict(xpool=xpool, dpool=dpool, gpool=gpool, spool=spool, qpool=qpool,
                     neg1=neg1_t)

        def body():
            st = _PipeState()
            # Input DMAs are prefetched one oct ahead and emitted before any
            # output DMA so the SP FIFO never holds the next oct's activations
            # behind store DMAs that wait on late compute (PE would starve at
            # oct boundaries and re-trigger the HAM throttle).
            prefetch(nc, pools, xw, det, 0, st)
            for o in range(n_oct):
                if o + 1 < n_oct:
                    prefetch(nc, pools, xw, det, o + 1, st)
                oct_body(nc, pools, w_t, xw, det, out, o, st)
            flush(nc, out, st)

        if loop > 1:
            with tc.For_i(0, loop):
                body()
        else:
            body()

    nc.finalize()
    return nc


class _PipeState:
    """Deferred pair-sections for software pipelining across emission."""

    def __init__(self):
        self.stage_b = []  # quads awaiting tanh + pool d1/d2
        self.stage_c = []  # quads awaiting final add + store
        self.xdet = {}     # oct -> (x_t, det_t) prefetched input tiles
        self.cur_tt1 = None
        self.quad_upd = []


def prefetch(nc, pools, xw, det, o, st):
    """DMA oct o's activations + deter slab into SBUF (one oct ahead)."""
    # ACT's HWDGE queue: input DMAs only ever wait on buffer-free (satisfied
    # long before issue), so they never stall the ACT sequencer — while the
    # store DMAs (which do wait on late compute) live on the SP queue where
    # nothing queues behind them.
    x_t = pools["xpool"].tile([128, 4, OCT], _BF16, name=f"x_{o}", tag="x")
    nc.scalar.dma_start(x_t[:], xw[:, :, o * OCT:(o + 1) * OCT])
    det_t = pools["dpool"].tile([128, 8, DETER], _BF16, name=f"dt_{o}", tag="det")
    nc.scalar.dma_start(
        det_t[:], det[o * OCT:(o + 1) * OCT, :].rearrange("(t p) f -> p t f", p=128)
    )
    st.xdet[o] = (x_t, det_t)


def oct_body(nc, pools, w_t, xw, det, out, o, st):
    """One oct (1024 batch rows = 4 pair-sections of 2x128 rows, grouped into
    2 quads for the tail stages)."""
    gpool, spool, qpool = pools["gpool"], pools["spool"], pools["qpool"]
    x_t, det_t = st.xdet.pop(o)

    for p in range(4):
        q = p // 2
        if p % 2 == 0:
            # quad-granular tt1 (tanh input) so the tanh/pool/store stages run
            # at N=1024 and amortize their per-instruction fixed costs
            tt1 = gpool.tile([128, 4, DETER], _BF16, name=f"tt1_{o}_{q}", tag="tt1")
            st.cur_tt1 = tt1
            # stage B of the previous quad: tanh FIRST in the section so the
            # ACT engine consumes it before stalling on this pair's rstd
            if st.stage_b:
                emit_tanh(nc, st.stage_b[0])
        tt1 = st.cur_tt1

        # ---- stage A(p): two fully tile-local subsections. Tile j's stats +
        # rstd + gates run while tile j+1's matmuls stream, so each tile's
        # PSUM banks free ~one tile-time after its matmuls — the 4-tile
        # rotation then never blocks the PE.
        #
        # st6 holds 3 bn groups of 256 per tile: [0:6] = bn_stats of q[0:512]
        # (even/odd 256-groups), [6:9] = a synthetic (count=256, mean=0,
        # M2=sum(q_u^2)) group whose M2 slot is filled by ACT Square's
        # accum_out. One bn_aggr then merges all three equal-count groups:
        # since the folded W gives mean(q)==0, its var output IS mean(q^2).
        st6 = spool.tile([128, 2, 9], _F32, name=f"st6_{o}_{p}", tag="st6")
        mv = spool.tile([128, 2, 2], _F32, name=f"mv_{o}_{p}", tag="mv")
        sqs = gpool.tile([128, 2, SQ_N], _BF16, name=f"sqs_{o}_{p}", tag="sqs")
        y1 = spool.tile([128, 2], _F32, name=f"y1_{o}_{p}", tag="y1")
        t2 = spool.tile([128, 2], _F32, name=f"t2_{o}_{p}", tag="t2")
        rstd = spool.tile([128, 2], _F32, name=f"rstd_{o}_{p}", tag="rstd")
        sig_r = gpool.tile([128, 2, DETER], _BF16, name=f"sig_{o}_{p}", tag="sig")
        upd = gpool.tile([128, 2, DETER], _BF16, name=f"upd_{o}_{p}", tag="upd")
        if o == 0:
            # slots 6 (count) and 7 (mean) are constants; the pool buffers
            # rotate with period 4 so writing them for o==0's four sections
            # initializes every physical buffer once for the whole pass
            nc.vector.memset(st6[:, :, 6], float(SQ_N))
            nc.vector.memset(st6[:, :, 7], 0.0)
        for j in range(2):
            t = 2 * p + j
            qt = qpool.tile([128, NOUT], _F32, name=f"q{o}_{t}", tag="q")  # 2 banks
            lhs_cols = slice(t * 128, (t + 1) * 128)
            # region-major order: the [0:512] bank finishes 4 matmuls earlier,
            # letting bn_stats (the longest stats op) start sooner
            for (qo, n) in _CHUNKS:
                for k in range(4):
                    nc.tensor.matmul(
                        qt[:, qo:qo + n],
                        x_t[:, k, lhs_cols],
                        w_t[:, k, qo:qo + n],
                        start=(k == 0),
                        stop=(k == 3),
                    )
                if qo == 0:
                    nc.vector.bn_stats(st6[:, j, 0:6], qt[:, 0:BN_N])
            nc.scalar.activation(
                out=sqs[:, j], in_=qt[:, BN_N:NOUT],
                func=mybir.ActivationFunctionType.Square,
                accum_out=st6[:, j, 8:9],
            )
            # rstd: v = mean(q^2) = bn_aggr's var, then 2-step Newton rsqrt
            # from y0=1 (W is normalized so v ~= 1; LN_EPS=1e-5 is negligible
            # against v's ~0.6 floor and is dropped):
            #   y1 = 1.5 - 0.5 v;  rstd = y1*(1.5 - 0.5*v*y1^2)
            jj = slice(j, j + 1)
            var = mv[:, jj, 1]
            nc.vector.bn_aggr(mv[:, j, :], st6[:, j, :])
            nc.vector.tensor_scalar(
                out=y1[:, jj], in0=var, scalar1=-0.5, scalar2=1.5,
                op0=mybir.AluOpType.mult, op1=mybir.AluOpType.add,
            )
            nc.vector.tensor_mul(t2[:, jj], y1[:, jj], y1[:, jj])    # y1^2
            nc.vector.tensor_mul(t2[:, jj], var, t2[:, jj])          # v*y1^2
            nc.vector.tensor_scalar(
                out=t2[:, jj], in0=t2[:, jj], scalar1=-0.5, scalar2=1.5,
                op0=mybir.AluOpType.mult, op1=mybir.AluOpType.add,
            )
            nc.vector.tensor_mul(rstd[:, jj], t2[:, jj], y1[:, jj])
            r_ = rstd[:, j:j + 1]
            nc.scalar.activation(
                out=sig_r[:, j], in_=qt[:, 0:DETER],
                func=mybir.ActivationFunctionType.Sigmoid, scale=r_,
            )
            nc.scalar.activation(
                out=upd[:, j], in_=qt[:, 2 * DETER:3 * DETER],
                func=mybir.ActivationFunctionType.Sigmoid, scale=r_,
                bias=pools["neg1"][:],
            )
            nc.vector.scalar_tensor_tensor(
                out=tt1[:, 2 * (p % 2) + j], in0=qt[:, DETER:2 * DETER],
                scalar=r_, in1=sig_r[:, j],
                op0=mybir.AluOpType.mult, op1=mybir.AluOpType.mult,
            )                                                        # (q_c*rstd)*sig_r

            # interleave the deferred tail stages between the two subsections
            if j == 0:
                if p % 2 == 0 and st.stage_b:
                    s = st.stage_b.pop(0)
                    emit_pool(nc, s)
                    st.stage_c.append(s)
                if p % 2 == 1 and st.stage_c:
                    emit_out(nc, out, st.stage_c.pop(0))

        if p % 2 == 0:
            st.quad_upd = [upd]
        else:
            st.quad_upd.append(upd)
            st.stage_b.append(dict(
                o=o, q=q, tt1=tt1, upds=st.quad_upd, det_t=det_t, gpool=gpool))

    # oct ends; stages drain via subsequent sections (or flush at the end)


def emit_tanh(nc, s):
    gpool = s["gpool"]
    cand = gpool.tile([128, 4, DETER], _BF16, name=f"cand_{s['o']}_{s['q']}", tag="cand")
    s["cand"] = cand
    nc.scalar.activation(
        out=cand[:], in_=s["tt1"][:], func=mybir.ActivationFunctionType.Tanh,
    )


def emit_pool(nc, s):
    gpool = s["gpool"]
    q = s["q"]
    d1 = gpool.tile([128, 4, DETER], _BF16, name=f"d1_{s['o']}_{q}", tag="d1")
    d2 = gpool.tile([128, 4, DETER], _BF16, name=f"d2_{s['o']}_{q}", tag="d2")
    dslc = s["det_t"][:, 4 * q:4 * q + 4]
    nc.gpsimd.tensor_sub(d1[:], s["cand"][:], dslc)                  # cand - det
    upds = s["upds"]
    nc.gpsimd.tensor_mul(d2[:, 0:2], upds[0][:], d1[:, 0:2])         # upd * (cand - det)
    nc.gpsimd.tensor_mul(d2[:, 2:4], upds[1][:], d1[:, 2:4])
    s["d2"] = d2
    s["dslc"] = dslc


def emit_out(nc, out, s):
    gpool = s["gpool"]
    o, q = s["o"], s["q"]
    outb = gpool.tile([128, 4, DETER], _BF16, name=f"outb_{o}_{q}", tag="outb")
    nc.vector.tensor_add(outb[:], s["d2"][:], s["dslc"])             # + det
    base = o * OCT + q * 512
    nc.sync.dma_start(
        out[base:base + 512, :].rearrange("(t p) f -> p t f", p=128),
        outb[:],
    )


def flush(nc, out, st):
    """Drain the deferred pipeline stages at the end of the program."""
    while st.stage_b or st.stage_c:
        if st.stage_b:
            s = st.stage_b.pop(0)
            emit_tanh(nc, s)
            emit_pool(nc, s)
            st.stage_c.append(s)
        if st.stage_c:
            emit_out(nc, out, st.stage_c.pop(0))


_nc_cache: dict[tuple, bass.Bass] = {}


def _get_nc(bc: int) -> bass.Bass:
    if (bc,) not in _nc_cache:
        _nc_cache[(bc,)] = build_nc(bc)
    return _nc_cache[(bc,)]


def _fold_weights(proj_w, proj_b, core_w):
    """Collapse both matmuls + LN mean-subtraction into one [KPAD, 768] matrix,
    Frobenius-normalized so mean over the 768 outputs of q^2 is ~1."""
    W1 = proj_w.astype(np.float64) @ core_w[:HID].astype(np.float64)   # [144, 768]
    W2 = core_w[HID:].astype(np.float64)                               # [256, 768]
    b1 = proj_b.astype(np.float64) @ core_w[:HID].astype(np.float64)   # [768]
    W_all = np.zeros((KPAD, NOUT), np.float64)
    W_all[0:STOCH] = W1[:STOCH]
    W_all[STOCH:STOCH + DETER] = W2
    W_all[STOCH + DETER:STOCH + DETER + ACT_DIM] = W1[STOCH:]
    W_all[STOCH + DETER + ACT_DIM] = b1
    # remove per-row column mean -> mean_j(x @ W_c) == 0 exactly
    W_c = W_all - W_all.mean(axis=1, keepdims=True)
    # normalize so E[mean_j q_j^2] == 1 (Newton rsqrt converges from y0=1;
    # LayerNorm output is invariant to this scale)
    W_c *= np.sqrt(NOUT / np.square(W_c).sum())
    return W_c


def kernel(deter, stoch, action, proj_w, proj_b, core_w, ln_g, ln_b):
    global _last_results
    deter = np.asarray(deter, np.float32)
    stoch = np.asarray(stoch, np.float32)
    action = np.asarray(action, np.float32)
    proj_w = np.asarray(proj_w, np.float32)
    proj_b = np.asarray(proj_b, np.float32)
    core_w = np.asarray(core_w, np.float32)
    ln_g = np.asarray(ln_g, np.float32)
    ln_b = np.asarray(ln_b, np.float32)

    if not (np.allclose(ln_g, 1.0) and np.allclose(ln_b, 0.0)):
        # General-affine LN is not wired into the device fast path; fall back to
        # exact host math (setup_inputs always passes g=1, b=0 so this is unused).
        return _host_reference(deter, stoch, action, proj_w, proj_b, core_w, ln_g, ln_b)

    B = deter.shape[0]
    assert B % N_CORES == 0
    bc = B // N_CORES

    W_c = _fold_weights(proj_w, proj_b, core_w)
    wp = np.ascontiguousarray(
        W_c.reshape(4, 128, NOUT).transpose(1, 0, 2)).astype(BF16)  # [128, 4, 768]

    # Feature-major activations, padded to KPAD rows: [stoch; deter; action; ones; zeros]
    xb = np.empty((KPAD, B), BF16)
    xb[0:STOCH] = stoch.T
    xb[STOCH:STOCH + DETER] = deter.T
    xb[STOCH + DETER:STOCH + DETER + ACT_DIM] = action.T
    xb[STOCH + DETER + ACT_DIM] = 1.0
    xb[STOCH + DETER + ACT_DIM + 1:] = 0.0
    xb = np.ascontiguousarray(xb.reshape(4, 128, B).transpose(1, 0, 2))  # [128, 4, B]

    det_b = deter.astype(BF16)

    in_maps = []
    for c in range(N_CORES):
        in_maps.append({
            "xw": np.ascontiguousarray(xb[:, :, c * bc:(c + 1) * bc]),
            "wts": wp,
            "det": np.ascontiguousarray(det_b[c * bc:(c + 1) * bc]),
        })

    nc = _get_nc(bc)
    res = run_bass_kernel_spmd(nc, in_maps, core_ids=list(range(N_CORES)))
    _last_results = res
    return np.concatenate(
        [res.results[c]["out"] for c in range(N_CORES)], axis=0
    ).astype(np.float32)


def _host_reference(deter, stoch, action, proj_w, proj_b, core_w, ln_g, ln_b):
    x = np.concatenate([stoch, action], axis=-1) @ proj_w + proj_b
    parts = np.concatenate([x, deter], axis=-1) @ core_w
    mu = parts.mean(-1, keepdims=True)
    var = ((parts - mu) ** 2).mean(-1, keepdims=True)
    parts = (parts - mu) / np.sqrt(var + LN_EPS) * ln_g + ln_b
    d = parts.shape[-1] // 3
    reset = 1.0 / (1.0 + np.exp(-parts[..., :d]))
    cand = np.tanh(reset * parts[..., d:2 * d])
    upd = 1.0 / (1.0 + np.exp(-(parts[..., 2 * d:] - 1.0)))
    return (upd * cand + (1.0 - upd) * deter).astype(np.float32)
